# revision 2
# baseline (speedup 1.0000x reference)
"""Trainium2 Bass kernel for the custom transformer layer (v2).

Sharding: 8 cores = 4 batches x 2 query-row halves (as v1).

v2 changes vs v1:
- Q/K/V projections and ptm logits run in fp8e4 (e4m3) with DoubleRow perf
  mode: K=256 contracted per instruction at 1 cycle/row -> ~1.8x tensor-engine
  throughput on phase C. Weights are pre-scaled by 64 on the host (descale is
  folded into the PSUM-evacuation activations). FFN stays bf16: fp8 there
  pushes rel-err over the 2e-2 gate (measured 0.020-0.031 in simulation).
- The attention bias is no longer added via identity matmuls on the PE.
  Instead phase B produces expb = exp(bias_scale*tanh(u) + mask) and phase D
  computes probs = exp(0.125*scores) * expb with the multiply on the idle
  vector engine. Saves ~82k PE cycles and removes the PSUM round trip.
- Scores for two mc-chunks accumulate into one [128,1024] PSUM span (two
  banks), halving the ACT instruction count for the exp.
- LayerNorm gamma/beta are folded into Wf1/bf1 on the host; the x2 residual
  term is reconstructed on gpsimd during phase E.
- h arrives in bf16 (halves the DMA) and all transposes run in bf16.
- h-chunk DMAs are issued before any weight prefetch so phase A starts early.
"""

import sys

sys.path.insert(0, "/opt/trn_rl_repo")

import numpy as np
import ml_dtypes

import concourse.bass as bass
import concourse.tile as tile
from concourse import bacc, mybir
from concourse.bass_utils import run_bass_kernel_spmd
from concourse.masks import make_identity

BF16 = ml_dtypes.bfloat16
FP8 = ml_dtypes.float8_e4m3
F32 = mybir.dt.float32
BF = mybir.dt.bfloat16
F8 = mybir.dt.float8e4
AF = mybir.ActivationFunctionType
ALU = mybir.AluOpType
DR = mybir.MatmulPerfMode.DoubleRow

B, S, H, NH, DH, I, C, P = 4, 1024, 1280, 20, 64, 5120, 13, 8
SQ = 512          # query rows per core
KO = H // 128     # 10 k-chunks
KO2 = KO // 2     # 5 fp8 pair-chunks
IC = I // 128     # 40 i-chunks
LC = SQ // 128    # 4 l-chunks
MC = S // 128     # 8 m-chunks
LN_EPS = 1e-5
WSCALE = 64.0     # fp8 weight pre-scale


def bcast_ap(src: bass.AP, parts: int) -> bass.AP:
    """Partition-stride-0 broadcast AP (for DMA sources)."""
    return bass.AP(tensor=src.tensor, offset=src.offset,
                   ap=[[0, parts]] + [list(d) for d in src.ap])


def build_nc(use_mask: bool, bias_scale: float):
    nc = bacc.Bacc("TRN2", target_bir_lowering=False, debug=False, num_devices=8)

    # ---- DRAM parameters (per-core) ----
    h_d = nc.declare_dram_parameter("h", [S, H], F8, isOutput=False)
    hres_d = nc.declare_dram_parameter("hres", [SQ, H], F32, isOutput=False)
    # fp8 QKV weights: [p, jc, j, i, m] = W^T[(2j+i)*128+p, jc*128+m] * 64
    wq8_d = nc.declare_dram_parameter("wq8", [128, KO, KO2, 2, 128], F8,
                                      isOutput=False)
    wk8_d = nc.declare_dram_parameter("wk8", [128, KO, KO2, 2, 128], F8,
                                      isOutput=False)
    # V moving: [p, j, i, vh] = Wv^T[(2j+i)*128+p, vh] * 64
    wv8_d = nc.declare_dram_parameter("wv8", [128, KO2, 2, H], F8, isOutput=False)
    # ptm stationary, zero-padded to 128 cols: [p, j, i, m] (m<P real)
    wptm8_d = nc.declare_dram_parameter("wptm8", [128, KO2, 2, 128], F8,
                                        isOutput=False)
    rmat_d = nc.declare_dram_parameter("rmat", [P, P], BF, isOutput=False)
    wf1T_d = nc.declare_dram_parameter("wf1T", [IC // 2, 128, KO, 256], BF,
                                       isOutput=False)
    wf2T_d = nc.declare_dram_parameter("wf2T", [IC, 128, H], BF, isOutput=False)
    bq_d = nc.declare_dram_parameter("bq", [128, KO], F32, isOutput=False)
    bk_d = nc.declare_dram_parameter("bk", [128, KO], F32, isOutput=False)
    bptm_d = nc.declare_dram_parameter("bptm", [P, 1], F32, isOutput=False)
    bf1_d = nc.declare_dram_parameter("bf1", [128, IC], F32, isOutput=False)
    lng_d = nc.declare_dram_parameter("lng", [128, H], F32, isOutput=False)
    lnbf_d = nc.declare_dram_parameter("lnbf", [128, H], F32, isOutput=False)
    mb_d = nc.declare_dram_parameter("mb", [128, MC], F32, isOutput=False)
    out_d = nc.declare_dram_parameter("out", [SQ, H], F32, isOutput=True)

    from contextlib import ExitStack
    with tile.TileContext(nc) as tc, ExitStack() as es:
        # ---- h chunks first: phase A can start as soon as chunk 0 lands ----
        es_hb = ExitStack()
        p_hb = es_hb.enter_context(tc.tile_pool(name="p_hb", bufs=8, side="right"))
        hb_tiles = []
        for sc in range(MC):
            hb = p_hb.tile([128, H], F8, tag="hb", name=f"hb_{sc}")
            heng = (nc.sync, nc.scalar, nc.gpsimd)[sc % 3]
            heng.dma_start(out=hb, in_=h_d.ap()[sc * 128:(sc + 1) * 128, :])
            hb_tiles.append(hb)

        # ---- persistent constants ----
        const = es.enter_context(tc.tile_pool(name="const", bufs=1))
        ident_b = const.tile([128, 128], BF)
        make_identity(nc, ident_b)
        ident8 = const.tile([128, 128], F8)
        make_identity(nc, ident8)
        bq_s = const.tile([128, KO], F32)
        nc.sync.dma_start(out=bq_s, in_=bq_d.ap())
        bk_s = const.tile([128, KO], F32)
        nc.sync.dma_start(out=bk_s, in_=bk_d.ap())
        bf1_s = const.tile([128, IC], F32)
        nc.sync.dma_start(out=bf1_s, in_=bf1_d.ap())
        bptm_s = const.tile([P, 1], F32)
        nc.sync.dma_start(out=bptm_s, in_=bptm_d.ap())
        mb_s = const.tile([128, MC], F32)
        nc.sync.dma_start(out=mb_s, in_=mb_d.ap())
        eps_s = const.tile([128, 1], F32)
        nc.vector.memset(eps_s, LN_EPS)
        ones8_s = const.tile([P, 1], BF)
        nc.vector.memset(ones8_s, 1.0)
        ones18_s = const.tile([1, P], BF)
        nc.vector.memset(ones18_s, 1.0)
        rmat_s = const.tile([P, P], BF)
        nc.sync.dma_start(out=rmat_s, in_=rmat_d.ap())
        wptm8_s = const.tile([128, KO2, 2, 128], F8)
        nc.sync.dma_start(out=wptm8_s, in_=wptm8_d.ap())
        lng_b = const.tile([128, H], F32)
        lnbf_b = const.tile([128, H], F32)

        # ---- QKV fp8 weights, prefetched after h ----
        es_w = ExitStack()
        p_w = es_w.enter_context(tc.tile_pool(name="p_w", bufs=1, side="right"))
        wq8_s = p_w.tile([128, KO, KO2, 2, 128], F8)
        wk8_s = p_w.tile([128, KO, KO2, 2, 128], F8)
        wv8_s = p_w.tile([128, KO2, 2, H], F8)
        for wt_s, wt_d in ((wq8_s, wq8_d), (wk8_s, wk8_d), (wv8_s, wv8_d)):
            nc.gpsimd.dma_start(out=wt_s, in_=wt_d.ap())

        es_hT = ExitStack()       # phases A..C
        p_hT = es_hT.enter_context(tc.tile_pool(name="p_hT", bufs=1, side="right"))
        hT8_s = p_hT.tile([128, KO2, 2, S], F8)        # h^T in fp8 pairs
        hTv_s = p_hT.tile([128, KO2, MC, 2, 128], F8)  # V stationary (contig pairs)
        es_attn = ExitStack()     # phases B..D
        p_attn = es_attn.enter_context(tc.tile_pool(name="p_attn", bufs=1))
        expb_s = p_attn.tile([128, MC, SQ], BF)        # exp(attention bias)
        QT_s = p_attn.tile([128, KO, SQ], BF)          # q^T (unscaled)
        KT_s = p_attn.tile([128, KO, S], BF)           # k^T
        # V with a ones column per head: [s-part, s-chunk, head, 64+1]
        vaug_s = p_attn.tile([128, MC, NH, DH + 1], BF)

        # ================= Phase A: h -> hT8 (fp8) =================
        with tc.tile_pool(name="ph_a_ps", bufs=6, space="PSUM") as paps:
            for sc in range(MC):
                for ko in range(KO):
                    # fp8 transpose requires output element step of 2
                    tp = paps.tile([128, 256], F8)
                    nc.tensor.transpose(tp[:, 0:256:2],
                                        hb_tiles[sc][:, ko * 128:(ko + 1) * 128],
                                        ident8)
                    dst = hT8_s[:, ko // 2, ko % 2, sc * 128:(sc + 1) * 128]
                    if (sc * KO + ko) % 2 == 0:
                        nc.vector.tensor_copy(out=dst, in_=tp[:, 0:256:2])
                    else:
                        nc.scalar.copy(out=dst, in_=tp[:, 0:256:2])

        # ========== Phases C+B interleaved: QKV (fp8 DR) + ptm bias ==========
        # B's serial chain (logits->exp->Z->recip->bcast->g->u->tanh->exp) is
        # spliced between C's long matmul streams so each dependency hop
        # drains while the PE streams Q/K/V, avoiding head-of-line blocking.
        nc.vector.memset(vaug_s[:, :, :, DH:DH + 1], 1.0)
        # V stationary needs contiguous fp8 pairs; derive from hT8 on-chip
        for j in range(KO2):
            for i in range(2):
                eng = nc.scalar if (2 * j + i) % 2 == 0 else nc.sync
                eng.dma_start(out=hTv_s[:, j, :, i, :],
                              in_=hT8_s[:, j, i, :].rearrange(
                                  "p (sc m) -> p sc m", m=128))
        with tc.tile_pool(name="ph_c", bufs=2) as pb, \
             tc.tile_pool(name="ph_c_big", bufs=1) as pbb, \
             tc.tile_pool(name="ph_c_ps", bufs=4, space="PSUM") as pcps, \
             tc.tile_pool(name="ph_b_lps", bufs=1, space="PSUM") as pblps, \
             tc.tile_pool(name="ph_b_zps", bufs=2, space="PSUM") as pbzps, \
             tc.tile_pool(name="ph_b_ups", bufs=1, space="PSUM") as pbups:
            # --- B: ptm logits + softmax numerator ---
            expT_s = pbb.tile([P, S], F32)
            for nb in range(4):
                lp = pblps.tile([128, 256], F32, tag="logits")
                for j in range(KO2):
                    nc.tensor.matmul(lp, wptm8_s[:, j, :, :],
                                     hT8_s[:, j, :, nb * 256:(nb + 1) * 256],
                                     start=(j == 0), stop=(j == KO2 - 1),
                                     perf_mode=DR)
                nc.scalar.activation(out=expT_s[:, nb * 256:(nb + 1) * 256],
                                     in_=lp[0:P, :], func=AF.Exp, bias=bptm_s,
                                     scale=1.0 / WSCALE)
            expTb_s = pbb.tile([P, S], BF)
            nc.vector.tensor_copy(out=expTb_s, in_=expT_s)

            # --- C: Q^T (own 512 rows), unscaled; exp applies 1/8 later ---
            for jc in range(KO):
                for nb in range(2):
                    qp = pcps.tile([128, 256], F32, tag="c",
                                   name=f"qp_{jc}_{nb}")
                    for j in range(KO2):
                        nc.tensor.matmul(qp, wq8_s[:, jc, j, :, :],
                                         hT8_s[:, j, :, nb * 256:(nb + 1) * 256],
                                         start=(j == 0), stop=(j == KO2 - 1),
                                         perf_mode=DR)
                    if (jc * 2 + nb) % 2 == 0:
                        nc.vector.tensor_scalar(
                            out=QT_s[:, jc, nb * 256:(nb + 1) * 256], in0=qp,
                            scalar1=1.0 / WSCALE, scalar2=bq_s[:, jc:jc + 1],
                            op0=ALU.mult, op1=ALU.add)
                    else:
                        nc.scalar.activation(
                            out=QT_s[:, jc, nb * 256:(nb + 1) * 256], in_=qp,
                            func=AF.Identity, bias=bq_s[:, jc:jc + 1],
                            scale=1.0 / WSCALE)

            # --- B: softmax normalization via matmul broadcast ---
            rz = pb.tile([1, S], F32, tag="rz")
            rzb = pb.tile([1, S], BF, tag="rzb")
            for n2 in range(2):
                zp = pbzps.tile([P, 512], F32, tag="zz", name=f"zp_{n2}")
                nc.tensor.matmul(zp[0:1, :], ones8_s,
                                 expTb_s[:, n2 * 512:(n2 + 1) * 512],
                                 start=True, stop=True)
                nc.vector.reciprocal(out=rz[:, n2 * 512:(n2 + 1) * 512],
                                     in_=zp[0:1, :])
            nc.vector.tensor_copy(out=rzb, in_=rz)
            ptmT_s = pbb.tile([P, S], BF)
            for n2 in range(2):
                zbp = pbzps.tile([P, 512], F32, tag="zz", name=f"zbp_{n2}")
                nc.tensor.matmul(zbp, ones18_s, rzb[:, n2 * 512:(n2 + 1) * 512],
                                 start=True, stop=True)
                nc.vector.tensor_mul(out=ptmT_s[:, n2 * 512:(n2 + 1) * 512],
                                     in0=expT_s[:, n2 * 512:(n2 + 1) * 512],
                                     in1=zbp)

            # --- C: K^T (all 1024 rows) ---
            for jc in range(KO):
                for nb in range(4):
                    kp = pcps.tile([128, 256], F32, tag="c",
                                   name=f"kp_{jc}_{nb}")
                    for j in range(KO2):
                        nc.tensor.matmul(kp, wk8_s[:, jc, j, :, :],
                                         hT8_s[:, j, :, nb * 256:(nb + 1) * 256],
                                         start=(j == 0), stop=(j == KO2 - 1),
                                         perf_mode=DR)
                    if (jc * 4 + nb) % 2 == 0:
                        nc.vector.tensor_scalar(
                            out=KT_s[:, jc, nb * 256:(nb + 1) * 256], in0=kp,
                            scalar1=1.0 / WSCALE, scalar2=bk_s[:, jc:jc + 1],
                            op0=ALU.mult, op1=ALU.add)
                    else:
                        nc.scalar.activation(
                            out=KT_s[:, jc, nb * 256:(nb + 1) * 256], in_=kp,
                            func=AF.Identity, bias=bk_s[:, jc:jc + 1],
                            scale=1.0 / WSCALE)

            # --- B: g = R @ ptm^T ---
            gp = pbzps.tile([P, 512], F32, tag="zz", name="gp")
            nc.tensor.matmul(gp, rmat_s, ptmT_s[:, :SQ], start=True, stop=True)
            gTs = pbb.tile([P, SQ], BF)
            nc.vector.tensor_copy(out=gTs, in_=gp)

            # --- C: V natural layout (all 1024 rows); bv folded into hres ---
            for sc in range(MC):
                for nb in range(5):
                    vp = pcps.tile([128, 256], F32, tag="c",
                                   name=f"vp_{sc}_{nb}")
                    for j in range(KO2):
                        nc.tensor.matmul(vp,
                                         hTv_s[:, j, sc, :, :],
                                         wv8_s[:, j, :, nb * 256:(nb + 1) * 256],
                                         start=(j == 0), stop=(j == KO2 - 1),
                                         perf_mode=DR)
                    if (sc * 5 + nb) % 2 == 0:
                        nc.vector.tensor_scalar_mul(
                            out=vaug_s[:, sc, nb * 4:(nb + 1) * 4, 0:DH],
                            in0=vp.rearrange("p (h d) -> p h d", d=DH),
                            scalar1=1.0 / WSCALE)
                    else:
                        nc.scalar.activation(
                            out=vaug_s[:, sc, nb * 4:(nb + 1) * 4, 0:DH],
                            in_=vp.rearrange("p (h d) -> p h d", d=DH),
                            func=AF.Copy, scale=1.0 / WSCALE)

            # --- B: u = ptm^T g, expb = exp(bias_scale*tanh(u) + mask) ---
            for mc in range(MC):
                up = pbups.tile([128, SQ], F32, tag="u")
                nc.tensor.matmul(up, ptmT_s[:, mc * 128:(mc + 1) * 128], gTs,
                                 start=True, stop=True)
                tt = pb.tile([128, SQ], F32, tag="tanh")
                nc.scalar.activation(out=tt, in_=up, func=AF.Tanh)
                if use_mask:
                    nc.scalar.activation(out=expb_s[:, mc, :], in_=tt,
                                         func=AF.Exp, scale=bias_scale,
                                         bias=mb_s[:, mc:mc + 1])
                else:
                    nc.scalar.activation(out=expb_s[:, mc, :], in_=tt,
                                         func=AF.Exp, scale=bias_scale)

        es_hT.close()  # free h^T
        es_w.close()   # free QKV weights
        es_hb.close()  # free h chunks

        # ================= Phase D: attention =================
        es_ctx = ExitStack()      # phases D..E
        p_ctx = es_ctx.enter_context(tc.tile_pool(name="p_ctx", bufs=1, side="right"))
        ctxn_s = p_ctx.tile([128, LC, H], BF)          # attention out, natural
        hres_s = p_ctx.tile([128, LC, H], F32)         # residual, lands during D
        for lc in range(LC):
            eng = (nc.scalar, nc.sync)[lc % 2]
            eng.dma_start(out=hres_s[:, lc, :],
                          in_=hres_d.ap()[lc * 128:(lc + 1) * 128, :])
        nc.gpsimd.dma_start(out=lng_b, in_=lng_d.ap())
        nc.gpsimd.dma_start(out=lnbf_b, in_=lnbf_d.ap())
        with tc.tile_pool(name="ph_d", bufs=3) as pd, \
             tc.tile_pool(name="ph_d_et", bufs=3) as pet, \
             tc.tile_pool(name="ph_d_pr", bufs=2) as pdp, \
             tc.tile_pool(name="ph_d_ps", bufs=1, space="PSUM") as pdps, \
             tc.tile_pool(name="ph_d_pst", bufs=2, space="PSUM") as pdpst, \
             tc.tile_pool(name="ph_d_ps2", bufs=2, space="PSUM") as pdps2:
            for hp in range(NH // 2):
                ko = hp
                # 6/10 head-pairs add the bias on the PE (identity matmul into
                # PSUM, exp writes probs directly); 4/10 multiply exp(bias) on
                # the DVE. Balances PE vs DVE load in this phase.
                pe_path = False
                pts = [pdp.tile([128, MC, SQ], BF, tag=f"probsT{i}",
                                name=f"pt_{hp}_{i}") for i in range(2)]
                for mp in range(MC // 2):
                    # two mc-chunks share one 2-bank PSUM span per head
                    sps = [pdps.tile([128, 1024], F32, tag=f"sc{i}",
                                     name=f"sp_{hp}_{mp}_{i}") for i in range(2)]
                    for half in range(2):
                        mc = 2 * mp + half
                        for i in range(2):
                            p0 = i * DH
                            nc.tensor.matmul(
                                sps[i][:, half * 512:(half + 1) * 512],
                                KT_s[p0:p0 + DH, ko, mc * 128:(mc + 1) * 128],
                                QT_s[p0:p0 + DH, ko, :],
                                start=True, stop=not pe_path)
                        if pe_path:
                            for i in range(2):
                                nc.tensor.matmul(
                                    sps[i][:, half * 512:(half + 1) * 512],
                                    ident_b, biasT8_s[:, mc, :],
                                    start=False, stop=True)
                    for i in range(2):
                        if pe_path:
                            nc.scalar.activation(
                                out=pts[i][:, 2 * mp:2 * mp + 2, :].rearrange(
                                    "p a b -> p (a b)"),
                                in_=sps[i], func=AF.Exp, scale=0.125)
                        else:
                            et = pet.tile([128, 1024], BF, tag="et",
                                          name=f"et_{hp}_{mp}_{i}")
                            nc.scalar.activation(out=et, in_=sps[i],
                                                 func=AF.Exp, scale=0.125)
                            nc.vector.tensor_mul(
                                out=pts[i][:, 2 * mp:2 * mp + 2, :].rearrange(
                                    "p a b -> p (a b)"),
                                in0=et,
                                in1=expb_s[:, 2 * mp:2 * mp + 2, :].rearrange(
                                    "p a b -> p (a b)"))
                for i in range(2):
                    hh = 2 * hp + i
                    cp = pdps2.tile([DH + 1, SQ], F32, tag="cx",
                                    name=f"cp_{hh}")
                    for mc in range(MC):
                        nc.tensor.matmul(cp, vaug_s[:, mc, hh, :],
                                         pts[i][:, mc, :],
                                         start=(mc == 0), stop=(mc == MC - 1))
                    cs = pd.tile([DH + 1, SQ], BF, tag="cs", name=f"cs_{hh}")
                    nc.vector.tensor_copy(out=cs, in_=cp)
                    for lc in range(LC):
                        tp = pdpst.tile([128, DH + 1], BF, tag="ct",
                                        name=f"ct_{hh}_{lc}")
                        nc.tensor.transpose(tp, cs[:, lc * 128:(lc + 1) * 128],
                                            ident_b[:DH + 1, :DH + 1])
                        rc = pd.tile([128, 1], F32, tag="rc",
                                     name=f"rc_{hh}_{lc}")
                        nc.vector.reciprocal(out=rc, in_=tp[:, DH:DH + 1])
                        nc.vector.tensor_scalar_mul(
                            out=ctxn_s[:, lc, hh * DH:(hh + 1) * DH],
                            in0=tp[:, 0:DH], scalar1=rc)

        es_attn.close()  # free expb/QT/KT/V

        # ================= Phase E: residual + LN =================
        es_x = ExitStack()        # phases E..G
        p_x = es_x.enter_context(tc.tile_pool(name="p_x", bufs=1))
        xh_s = p_x.tile([128, LC, H], F32)             # standardized x
        x2_s = p_x.tile([128, LC, H], F32)             # xh*g + (ln_b + bf2)
        xT_s = p_x.tile([128, KO, SQ], BF)             # xh^T
        gT_s = p_x.tile([128, IC, SQ], BF)             # gelu(ffn1)^T
        with tc.tile_pool(name="ph_e", bufs=2) as pe, \
             tc.tile_pool(name="ph_e_ps", bufs=4, space="PSUM") as peps:
            for lc in range(LC):
                xs = xh_s[:, lc, :]
                nc.vector.tensor_add(out=xs, in0=hres_s[:, lc, :],
                                     in1=ctxn_s[:, lc, :])
                st = pe.tile([128, 5, 6], F32, tag="st")
                xg = xs.rearrange("p (g d) -> p g d", d=256)
                for sg in range(5):
                    nc.vector.bn_stats(out=st[:, sg, :], in_=xg[:, sg, :])
                mv = pe.tile([128, 2], F32, tag="mv")
                nc.vector.bn_aggr(out=mv, in_=st)
                sd = pe.tile([128, 1], F32, tag="sd")
                nc.scalar.activation(out=sd, in_=mv[:, 1:2], func=AF.Sqrt,
                                     bias=eps_s)
                rs = pe.tile([128, 1], F32, tag="rs")
                nc.vector.reciprocal(out=rs, in_=sd)
                nc.vector.tensor_scalar(out=xs, in0=xs, scalar1=mv[:, 0:1],
                                        scalar2=rs, op0=ALU.subtract, op1=ALU.mult)
                # xg = xh*g on gpsimd (idle here); lnbf added in phase G
                nc.gpsimd.tensor_mul(out=x2_s[:, lc, :], in0=xs, in1=lng_b)
                for ko in range(KO):
                    tpx = peps.tile([128, 128], BF, tag="xt")
                    xsb = pe.tile([128, 128], BF, tag="xsb")
                    nc.scalar.copy(out=xsb, in_=xs[:, ko * 128:(ko + 1) * 128])
                    nc.tensor.transpose(tpx, xsb, ident_b)
                    nc.vector.tensor_copy(
                        out=xT_s[:, ko, lc * 128:(lc + 1) * 128], in_=tpx)
        es_ctx.close()  # free ctxn

        # ================= Phase F: FFN1 (gelu) =================
        with tc.tile_pool(name="ph_f_w", bufs=8) as pfw, \
             tc.tile_pool(name="ph_f_ps", bufs=4, space="PSUM") as pfps:
            for ic2 in range(IC // 2):
                wt = pfw.tile([128, KO, 256], BF, tag="w1")
                eng = (nc.sync, nc.gpsimd, nc.scalar)[ic2 % 3]
                eng.dma_start(out=wt, in_=wf1T_d.ap()[ic2])
                for i_in in range(2):
                    ic = ic2 * 2 + i_in
                    yp = pfps.tile([128, SQ], F32, tag="y")
                    for ko in range(KO):
                        nc.tensor.matmul(yp,
                                         wt[:, ko, i_in * 128:(i_in + 1) * 128],
                                         xT_s[:, ko, :],
                                         start=(ko == 0), stop=(ko == KO - 1))
                    nc.scalar.activation(out=gT_s[:, ic, :], in_=yp, func=AF.Gelu,
                                         bias=bf1_s[:, ic:ic + 1])

        # ================= Phase G: FFN2 + residual + store =================
        with tc.tile_pool(name="ph_g_w", bufs=8) as pgw, \
             tc.tile_pool(name="ph_g_o", bufs=3) as pgo, \
             tc.tile_pool(name="ph_g_ps", bufs=2, space="PSUM") as pgps:
            for j0, jn in ((0, 512), (512, 512), (1024, 256)):
                zps = [pgps.tile([128, jn], F32, tag=f"z{lc}", name=f"zp_{j0}_{lc}")
                       for lc in range(LC)]
                for ic in range(IC):
                    w2 = pgw.tile([128, 512], BF, tag="w2")
                    eng = (nc.sync, nc.gpsimd, nc.scalar)[ic % 3]
                    eng.dma_start(out=w2[:, :jn],
                                  in_=wf2T_d.ap()[ic, :, j0:j0 + jn])
                    for lc in range(LC):
                        nc.tensor.matmul(zps[lc],
                                         gT_s[:, ic, lc * 128:(lc + 1) * 128],
                                         w2[:, :jn],
                                         start=(ic == 0), stop=(ic == IC - 1))
                for lc in range(LC):
                    ot = pgo.tile([128, 512], F32, tag="ot")
                    nc.vector.tensor_add(out=ot[:, :jn], in0=zps[lc],
                                         in1=x2_s[:, lc, j0:j0 + jn])
                    nc.vector.tensor_add(out=ot[:, :jn], in0=ot[:, :jn],
                                         in1=lnbf_b[:, j0:j0 + jn])
                    nc.sync.dma_start(
                        out=out_d.ap()[lc * 128:(lc + 1) * 128, j0:j0 + jn],
                        in_=ot[:, :jn])
        es_x.close()

    nc.compile()
    return nc


_NC_CACHE = {}


def _get_nc(use_mask: bool, bias_scale: float):
    key = (use_mask, round(bias_scale, 9))
    if key not in _NC_CACHE:
        _NC_CACHE[key] = build_nc(use_mask, bias_scale)
    return _NC_CACHE[key]


def _prep_inputs(inputs):
    f32 = lambda x: np.ascontiguousarray(np.asarray(x, np.float32))
    hs = f32(inputs["hidden_states"])
    mask = f32(inputs["attention_mask"])
    M, W1, b1, W2, b2 = (f32(inputs["M"]), f32(inputs["W_ct1"]),
                         f32(inputs["b_ct1"]), f32(inputs["W_ct2"]),
                         f32(inputs["b_ct2"]))
    R = ((M.T @ W1.T + b1).T @ (M @ W2.T + b2)).astype(np.float32)
    bias_scale = float(np.asarray(inputs["bias_scale"]).reshape(-1)[0])
    use_mask = not bool(np.all(mask == 1.0))

    def pack_fp8_pairs(wT, m_block):
        # (H, J) -> [128, J/m_block, KO2, 2, m_block] fp8, scaled by WSCALE
        Hh, J = wT.shape
        w = (wT * WSCALE).reshape(KO2, 2, 128, J // m_block, m_block)
        return np.ascontiguousarray(w.transpose(2, 3, 0, 1, 4)).astype(FP8)

    wqT, wkT, wvT = (f32(inputs["Wq"]).T, f32(inputs["Wk"]).T,
                     f32(inputs["Wv"]).T)
    wq8 = pack_fp8_pairs(wqT, 128)                # (128, 10, 5, 2, 128)
    wk8 = pack_fp8_pairs(wkT, 128)
    # V moving: [p, j, i, vh]
    wv8 = np.ascontiguousarray(
        (wvT * WSCALE).reshape(KO2, 2, 128, H).transpose(2, 0, 1, 3)).astype(FP8)
    wptmT = np.zeros((H, 128), np.float32)
    wptmT[:, :P] = f32(inputs["W_ptm"]).T
    wptm8 = pack_fp8_pairs(wptmT, 128)[:, 0]      # (128, 5, 2, 128), zero-padded

    lng = f32(inputs["ln_g"])
    lnb = f32(inputs["ln_b"])
    Wf1 = f32(inputs["Wf1"])
    wf1T = np.ascontiguousarray((Wf1 * lng[None, :]).T).astype(BF16)  # (H, I)
    wf1p = np.ascontiguousarray(
        wf1T.reshape(KO, 128, IC // 2, 256).transpose(2, 0, 1, 3)
            .transpose(0, 2, 1, 3))               # (IC//2, 128, KO, 256)
    wf2T = np.ascontiguousarray(f32(inputs["Wf2"]).T).astype(BF16)    # (I, H)
    wf2p = np.ascontiguousarray(wf2T.reshape(IC, 128, H))
    shared = {
        "wq8": wq8, "wk8": wk8, "wv8": wv8, "wptm8": wptm8,
        "rmat": np.ascontiguousarray(R).astype(BF16),
        "wf1T": wf1p, "wf2T": wf2p,
        "bq": np.ascontiguousarray(f32(inputs["bq"]).reshape(KO, 128).T),
        "bk": np.ascontiguousarray(f32(inputs["bk"]).reshape(KO, 128).T),
        "bptm": f32(inputs["b_ptm"]).reshape(P, 1),
        "bf1": np.ascontiguousarray(
            (f32(inputs["bf1"]) + Wf1 @ lnb).reshape(IC, 128).T),
        "lng": np.ascontiguousarray(np.broadcast_to(lng, (128, H))),
        "lnbf": np.ascontiguousarray(
            np.broadcast_to(lnb + f32(inputs["bf2"]), (128, H))),
    }
    bv = f32(inputs["bv"])
    in_maps = []
    for c in range(8):
        b, half = c // 2, c % 2
        r0 = half * SQ
        mb = np.roll((1.0 - mask[b]) * np.float32(-1e30), -r0)
        m = dict(shared)
        m["h"] = np.ascontiguousarray(np.roll(hs[b], -r0, axis=0)).astype(FP8)
        m["hres"] = np.ascontiguousarray(hs[b, r0:r0 + SQ] + bv[None, :])
        m["mb"] = np.ascontiguousarray(mb.reshape(MC, 128).T)
        in_maps.append(m)
    return in_maps, use_mask, bias_scale


def kernel(**inputs) -> np.ndarray:
    in_maps, use_mask, bias_scale = _prep_inputs(inputs)
    nc = _get_nc(use_mask, bias_scale)
    res = run_bass_kernel_spmd(nc, in_maps, list(range(8)))
    out = np.zeros((B, S, H), np.float32)
    for c in range(8):
        b, half = c // 2, c % 2
        r0 = half * SQ
        out[b, r0:r0 + SQ] = res.results[c]["out"]
    return out


# revision 3
# speedup vs baseline: 1.0380x; 1.0380x over previous
"""Trainium2 Bass kernel for the custom transformer layer (v2).

Sharding: 8 cores = 4 batches x 2 query-row halves (as v1).

v2 changes vs v1:
- Q/K/V projections and ptm logits run in fp8e4 (e4m3) with DoubleRow perf
  mode: K=256 contracted per instruction at 1 cycle/row -> ~1.8x tensor-engine
  throughput on phase C. Weights are pre-scaled by 64 on the host (descale is
  folded into the PSUM-evacuation activations). FFN stays bf16: fp8 there
  pushes rel-err over the 2e-2 gate (measured 0.020-0.031 in simulation).
- The attention bias is no longer added via identity matmuls on the PE.
  Instead phase B produces expb = exp(bias_scale*tanh(u) + mask) and phase D
  computes probs = exp(0.125*scores) * expb with the multiply on the idle
  vector engine. Saves ~82k PE cycles and removes the PSUM round trip.
- Scores for two mc-chunks accumulate into one [128,1024] PSUM span (two
  banks), halving the ACT instruction count for the exp.
- LayerNorm gamma/beta are folded into Wf1/bf1 on the host; the x2 residual
  term is reconstructed on gpsimd during phase E.
- h arrives in fp8 (quarters the DMA; hT8 is fp8 anyway) and the h
  transposes run in fp8 with the stride-2 PSUM output the ISA requires.
- h-chunk DMAs are issued before any weight prefetch so phase A starts early.
"""

import sys

sys.path.insert(0, "/opt/trn_rl_repo")

import numpy as np
import ml_dtypes

import concourse.bass as bass
import concourse.tile as tile
from concourse import bacc, mybir
from concourse.bass_utils import run_bass_kernel_spmd
from concourse.masks import make_identity

BF16 = ml_dtypes.bfloat16
FP8 = ml_dtypes.float8_e4m3
F32 = mybir.dt.float32
BF = mybir.dt.bfloat16
F8 = mybir.dt.float8e4
AF = mybir.ActivationFunctionType
ALU = mybir.AluOpType
DR = mybir.MatmulPerfMode.DoubleRow

B, S, H, NH, DH, I, C, P = 4, 1024, 1280, 20, 64, 5120, 13, 8
SQ = 512          # query rows per core
KO = H // 128     # 10 k-chunks
KO2 = KO // 2     # 5 fp8 pair-chunks
IC = I // 128     # 40 i-chunks
LC = SQ // 128    # 4 l-chunks
MC = S // 128     # 8 m-chunks
LN_EPS = 1e-5
WSCALE = 64.0     # fp8 weight pre-scale


def bcast_ap(src: bass.AP, parts: int) -> bass.AP:
    """Partition-stride-0 broadcast AP (for DMA sources)."""
    return bass.AP(tensor=src.tensor, offset=src.offset,
                   ap=[[0, parts]] + [list(d) for d in src.ap])


def build_nc(use_mask: bool, bias_scale: float):
    nc = bacc.Bacc("TRN2", target_bir_lowering=False, debug=False, num_devices=8)

    # ---- DRAM parameters (per-core) ----
    h_d = nc.declare_dram_parameter("h", [S, H], F8, isOutput=False)
    hres_d = nc.declare_dram_parameter("hres", [SQ, H], F32, isOutput=False)
    # fp8 QKV weights: [p, jc, j, i, m] = W^T[(2j+i)*128+p, jc*128+m] * 64
    wq8_d = nc.declare_dram_parameter("wq8", [128, KO, KO2, 2, 128], F8,
                                      isOutput=False)
    wk8_d = nc.declare_dram_parameter("wk8", [128, KO, KO2, 2, 128], F8,
                                      isOutput=False)
    # V moving: [p, j, i, vh] = Wv^T[(2j+i)*128+p, vh] * 64
    wv8_d = nc.declare_dram_parameter("wv8", [128, KO2, 2, H], F8, isOutput=False)
    # ptm stationary, zero-padded to 128 cols: [p, j, i, m] (m<P real)
    wptm8_d = nc.declare_dram_parameter("wptm8", [128, KO2, 2, 128], F8,
                                        isOutput=False)
    rmat_d = nc.declare_dram_parameter("rmat", [P, P], BF, isOutput=False)
    wf1T_d = nc.declare_dram_parameter("wf1T", [IC // 2, 128, KO, 256], BF,
                                       isOutput=False)
    wf2T_d = nc.declare_dram_parameter("wf2T", [IC, 128, H], BF, isOutput=False)
    bq_d = nc.declare_dram_parameter("bq", [128, KO], F32, isOutput=False)
    bk_d = nc.declare_dram_parameter("bk", [128, KO], F32, isOutput=False)
    bptm_d = nc.declare_dram_parameter("bptm", [P, 1], F32, isOutput=False)
    bf1_d = nc.declare_dram_parameter("bf1", [128, IC], F32, isOutput=False)
    lng_d = nc.declare_dram_parameter("lng", [128, H], F32, isOutput=False)
    lnbf_d = nc.declare_dram_parameter("lnbf", [128, H], F32, isOutput=False)
    mb_d = nc.declare_dram_parameter("mb", [128, MC], F32, isOutput=False)
    out_d = nc.declare_dram_parameter("out", [SQ, H], F32, isOutput=True)

    from contextlib import ExitStack
    with tile.TileContext(nc) as tc, ExitStack() as es:
        # ---- h chunks first: phase A can start as soon as chunk 0 lands ----
        es_hb = ExitStack()
        p_hb = es_hb.enter_context(tc.tile_pool(name="p_hb", bufs=8, side="right"))
        hb_tiles = []
        for sc in range(MC):
            hb = p_hb.tile([128, H], F8, tag="hb", name=f"hb_{sc}")
            heng = (nc.sync, nc.scalar, nc.gpsimd)[sc % 3]
            heng.dma_start(out=hb, in_=h_d.ap()[sc * 128:(sc + 1) * 128, :])
            hb_tiles.append(hb)

        # ---- persistent constants ----
        const = es.enter_context(tc.tile_pool(name="const", bufs=1))
        ident_b = const.tile([128, 128], BF)
        make_identity(nc, ident_b)
        ident8 = const.tile([128, 128], F8)
        make_identity(nc, ident8)
        bq_s = const.tile([128, KO], F32)
        nc.sync.dma_start(out=bq_s, in_=bq_d.ap())
        bk_s = const.tile([128, KO], F32)
        nc.sync.dma_start(out=bk_s, in_=bk_d.ap())
        bf1_s = const.tile([128, IC], F32)
        nc.sync.dma_start(out=bf1_s, in_=bf1_d.ap())
        bptm_s = const.tile([P, 1], F32)
        nc.sync.dma_start(out=bptm_s, in_=bptm_d.ap())
        mb_s = const.tile([128, MC], F32)
        nc.sync.dma_start(out=mb_s, in_=mb_d.ap())
        eps_s = const.tile([128, 1], F32)
        nc.vector.memset(eps_s, LN_EPS)
        ones8_s = const.tile([P, 1], BF)
        nc.vector.memset(ones8_s, 1.0)
        ones18_s = const.tile([1, P], BF)
        nc.vector.memset(ones18_s, 1.0)
        rmat_s = const.tile([P, P], BF)
        nc.sync.dma_start(out=rmat_s, in_=rmat_d.ap())
        wptm8_s = const.tile([128, KO2, 2, 128], F8)
        nc.sync.dma_start(out=wptm8_s, in_=wptm8_d.ap())
        lng_b = const.tile([128, H], F32)
        lnbf_b = const.tile([128, H], F32)

        # ---- QKV fp8 weights, prefetched after h ----
        es_w = ExitStack()
        p_w = es_w.enter_context(tc.tile_pool(name="p_w", bufs=1, side="right"))
        wq8_s = p_w.tile([128, KO, KO2, 2, 128], F8)
        wk8_s = p_w.tile([128, KO, KO2, 2, 128], F8)
        wv8_s = p_w.tile([128, KO2, 2, H], F8)
        for wt_s, wt_d in ((wq8_s, wq8_d), (wk8_s, wk8_d), (wv8_s, wv8_d)):
            nc.gpsimd.dma_start(out=wt_s, in_=wt_d.ap())

        es_hT = ExitStack()       # phases A..C
        p_hT = es_hT.enter_context(tc.tile_pool(name="p_hT", bufs=1, side="right"))
        hT8_s = p_hT.tile([128, KO2, 2, S], F8)        # h^T in fp8 pairs
        hTv_s = p_hT.tile([128, KO2, MC, 2, 128], F8)  # V stationary (contig pairs)
        es_attn = ExitStack()     # phases B..D
        p_attn = es_attn.enter_context(tc.tile_pool(name="p_attn", bufs=1))
        expb_s = p_attn.tile([128, MC, SQ], BF)        # exp(attention bias)
        QT_s = p_attn.tile([128, KO, SQ], BF)          # q^T (unscaled)
        KT_s = p_attn.tile([128, KO, S], BF)           # k^T
        # V with a ones column per head: [s-part, s-chunk, head, 64+1]
        vaug_s = p_attn.tile([128, MC, NH, DH + 1], BF)

        # ================= Phase A: h -> hT8 (fp8) =================
        with tc.tile_pool(name="ph_a_ps", bufs=6, space="PSUM") as paps:
            for sc in range(MC):
                for ko in range(KO):
                    # fp8 transpose requires output element step of 2
                    tp = paps.tile([128, 256], F8)
                    nc.tensor.transpose(tp[:, 0:256:2],
                                        hb_tiles[sc][:, ko * 128:(ko + 1) * 128],
                                        ident8)
                    dst = hT8_s[:, ko // 2, ko % 2, sc * 128:(sc + 1) * 128]
                    if (sc * KO + ko) % 2 == 0:
                        nc.vector.tensor_copy(out=dst, in_=tp[:, 0:256:2])
                    else:
                        nc.scalar.copy(out=dst, in_=tp[:, 0:256:2])

        # ========== Phases C+B interleaved: QKV (fp8 DR) + ptm bias ==========
        # B's serial chain (logits->exp->Z->recip->bcast->g->u->tanh->exp) is
        # spliced between C's long matmul streams so each dependency hop
        # drains while the PE streams Q/K/V, avoiding head-of-line blocking.
        nc.vector.memset(vaug_s[:, :, :, DH:DH + 1], 1.0)
        # V stationary needs contiguous fp8 pairs; derive from hT8 on-chip
        for j in range(KO2):
            for i in range(2):
                eng = nc.scalar if (2 * j + i) % 2 == 0 else nc.sync
                eng.dma_start(out=hTv_s[:, j, :, i, :],
                              in_=hT8_s[:, j, i, :].rearrange(
                                  "p (sc m) -> p sc m", m=128))
        with tc.tile_pool(name="ph_c", bufs=2) as pb, \
             tc.tile_pool(name="ph_c_big", bufs=1) as pbb, \
             tc.tile_pool(name="ph_c_ps", bufs=4, space="PSUM") as pcps, \
             tc.tile_pool(name="ph_b_lps", bufs=1, space="PSUM") as pblps, \
             tc.tile_pool(name="ph_b_zps", bufs=2, space="PSUM") as pbzps, \
             tc.tile_pool(name="ph_b_ups", bufs=1, space="PSUM") as pbups:
            # --- B: ptm logits + softmax numerator ---
            expT_s = pbb.tile([P, S], F32)
            for nb in range(4):
                lp = pblps.tile([128, 256], F32, tag="logits")
                for j in range(KO2):
                    nc.tensor.matmul(lp, wptm8_s[:, j, :, :],
                                     hT8_s[:, j, :, nb * 256:(nb + 1) * 256],
                                     start=(j == 0), stop=(j == KO2 - 1),
                                     perf_mode=DR)
                nc.scalar.activation(out=expT_s[:, nb * 256:(nb + 1) * 256],
                                     in_=lp[0:P, :], func=AF.Exp, bias=bptm_s,
                                     scale=1.0 / WSCALE)
            expTb_s = pbb.tile([P, S], BF)
            nc.vector.tensor_copy(out=expTb_s, in_=expT_s)

            # --- C: Q^T (own 512 rows), unscaled; exp applies 1/8 later ---
            for jc in range(KO):
                for nb in range(2):
                    qp = pcps.tile([128, 256], F32, tag="c",
                                   name=f"qp_{jc}_{nb}")
                    for j in range(KO2):
                        nc.tensor.matmul(qp, wq8_s[:, jc, j, :, :],
                                         hT8_s[:, j, :, nb * 256:(nb + 1) * 256],
                                         start=(j == 0), stop=(j == KO2 - 1),
                                         perf_mode=DR)
                    if (jc * 2 + nb) % 2 == 0:
                        nc.vector.tensor_scalar(
                            out=QT_s[:, jc, nb * 256:(nb + 1) * 256], in0=qp,
                            scalar1=1.0 / WSCALE, scalar2=bq_s[:, jc:jc + 1],
                            op0=ALU.mult, op1=ALU.add)
                    else:
                        nc.scalar.activation(
                            out=QT_s[:, jc, nb * 256:(nb + 1) * 256], in_=qp,
                            func=AF.Identity, bias=bq_s[:, jc:jc + 1],
                            scale=1.0 / WSCALE)

            # --- B: softmax normalization via matmul broadcast ---
            rz = pb.tile([1, S], F32, tag="rz")
            rzb = pb.tile([1, S], BF, tag="rzb")
            for n2 in range(2):
                zp = pbzps.tile([P, 512], F32, tag="zz", name=f"zp_{n2}")
                nc.tensor.matmul(zp[0:1, :], ones8_s,
                                 expTb_s[:, n2 * 512:(n2 + 1) * 512],
                                 start=True, stop=True)
                nc.vector.reciprocal(out=rz[:, n2 * 512:(n2 + 1) * 512],
                                     in_=zp[0:1, :])
            nc.vector.tensor_copy(out=rzb, in_=rz)
            ptmT_s = pbb.tile([P, S], BF)
            for n2 in range(2):
                zbp = pbzps.tile([P, 512], F32, tag="zz", name=f"zbp_{n2}")
                nc.tensor.matmul(zbp, ones18_s, rzb[:, n2 * 512:(n2 + 1) * 512],
                                 start=True, stop=True)
                nc.vector.tensor_mul(out=ptmT_s[:, n2 * 512:(n2 + 1) * 512],
                                     in0=expT_s[:, n2 * 512:(n2 + 1) * 512],
                                     in1=zbp)

            # --- C: K^T (all 1024 rows) ---
            for jc in range(KO):
                for nb in range(4):
                    kp = pcps.tile([128, 256], F32, tag="c",
                                   name=f"kp_{jc}_{nb}")
                    for j in range(KO2):
                        nc.tensor.matmul(kp, wk8_s[:, jc, j, :, :],
                                         hT8_s[:, j, :, nb * 256:(nb + 1) * 256],
                                         start=(j == 0), stop=(j == KO2 - 1),
                                         perf_mode=DR)
                    if (jc * 4 + nb) % 2 == 0:
                        nc.vector.tensor_scalar(
                            out=KT_s[:, jc, nb * 256:(nb + 1) * 256], in0=kp,
                            scalar1=1.0 / WSCALE, scalar2=bk_s[:, jc:jc + 1],
                            op0=ALU.mult, op1=ALU.add)
                    else:
                        nc.scalar.activation(
                            out=KT_s[:, jc, nb * 256:(nb + 1) * 256], in_=kp,
                            func=AF.Identity, bias=bk_s[:, jc:jc + 1],
                            scale=1.0 / WSCALE)

            # --- B: g = R @ ptm^T ---
            gp = pbzps.tile([P, 512], F32, tag="zz", name="gp")
            nc.tensor.matmul(gp, rmat_s, ptmT_s[:, :SQ], start=True, stop=True)
            gTs = pbb.tile([P, SQ], BF)
            nc.vector.tensor_copy(out=gTs, in_=gp)

            # --- C: V natural layout (all 1024 rows); bv folded into hres ---
            for sc in range(MC):
                for nb in range(5):
                    vp = pcps.tile([128, 256], F32, tag="c",
                                   name=f"vp_{sc}_{nb}")
                    for j in range(KO2):
                        nc.tensor.matmul(vp,
                                         hTv_s[:, j, sc, :, :],
                                         wv8_s[:, j, :, nb * 256:(nb + 1) * 256],
                                         start=(j == 0), stop=(j == KO2 - 1),
                                         perf_mode=DR)
                    if (sc * 5 + nb) % 2 == 0:
                        nc.vector.tensor_scalar_mul(
                            out=vaug_s[:, sc, nb * 4:(nb + 1) * 4, 0:DH],
                            in0=vp.rearrange("p (h d) -> p h d", d=DH),
                            scalar1=1.0 / WSCALE)
                    else:
                        nc.scalar.activation(
                            out=vaug_s[:, sc, nb * 4:(nb + 1) * 4, 0:DH],
                            in_=vp.rearrange("p (h d) -> p h d", d=DH),
                            func=AF.Copy, scale=1.0 / WSCALE)

            # --- B: u = ptm^T g, expb = exp(bias_scale*tanh(u) + mask) ---
            for mc in range(MC):
                up = pbups.tile([128, SQ], F32, tag="u")
                nc.tensor.matmul(up, ptmT_s[:, mc * 128:(mc + 1) * 128], gTs,
                                 start=True, stop=True)
                tt = pb.tile([128, SQ], F32, tag="tanh")
                nc.scalar.activation(out=tt, in_=up, func=AF.Tanh)
                if use_mask:
                    nc.scalar.activation(out=expb_s[:, mc, :], in_=tt,
                                         func=AF.Exp, scale=bias_scale,
                                         bias=mb_s[:, mc:mc + 1])
                else:
                    nc.scalar.activation(out=expb_s[:, mc, :], in_=tt,
                                         func=AF.Exp, scale=bias_scale)

        es_hT.close()  # free h^T
        es_w.close()   # free QKV weights
        es_hb.close()  # free h chunks

        # ================= Phase D: attention =================
        es_ctx = ExitStack()      # phases D..E
        p_ctx = es_ctx.enter_context(tc.tile_pool(name="p_ctx", bufs=1, side="right"))
        ctxn_s = p_ctx.tile([128, LC, H], BF)          # attention out, natural
        hres_s = p_ctx.tile([128, LC, H], F32)         # residual, lands during D
        for lc in range(LC):
            eng = (nc.scalar, nc.sync)[lc % 2]
            eng.dma_start(out=hres_s[:, lc, :],
                          in_=hres_d.ap()[lc * 128:(lc + 1) * 128, :])
        nc.gpsimd.dma_start(out=lng_b, in_=lng_d.ap())
        nc.gpsimd.dma_start(out=lnbf_b, in_=lnbf_d.ap())
        with tc.tile_pool(name="ph_d", bufs=3) as pd, \
             tc.tile_pool(name="ph_d_et", bufs=3) as pet, \
             tc.tile_pool(name="ph_d_pr", bufs=2) as pdp, \
             tc.tile_pool(name="ph_d_ps", bufs=1, space="PSUM") as pdps, \
             tc.tile_pool(name="ph_d_pst", bufs=2, space="PSUM") as pdpst, \
             tc.tile_pool(name="ph_d_ps2", bufs=2, space="PSUM") as pdps2:
            for hp in range(NH // 2):
                ko = hp
                # 6/10 head-pairs add the bias on the PE (identity matmul into
                # PSUM, exp writes probs directly); 4/10 multiply exp(bias) on
                # the DVE. Balances PE vs DVE load in this phase.
                pe_path = False
                pts = [pdp.tile([128, MC, SQ], BF, tag=f"probsT{i}",
                                name=f"pt_{hp}_{i}") for i in range(2)]
                for mp in range(MC // 2):
                    # two mc-chunks share one 2-bank PSUM span per head
                    sps = [pdps.tile([128, 1024], F32, tag=f"sc{i}",
                                     name=f"sp_{hp}_{mp}_{i}") for i in range(2)]
                    for half in range(2):
                        mc = 2 * mp + half
                        for i in range(2):
                            p0 = i * DH
                            nc.tensor.matmul(
                                sps[i][:, half * 512:(half + 1) * 512],
                                KT_s[p0:p0 + DH, ko, mc * 128:(mc + 1) * 128],
                                QT_s[p0:p0 + DH, ko, :],
                                start=True, stop=not pe_path)
                        if pe_path:
                            for i in range(2):
                                nc.tensor.matmul(
                                    sps[i][:, half * 512:(half + 1) * 512],
                                    ident_b, biasT8_s[:, mc, :],
                                    start=False, stop=True)
                    for i in range(2):
                        if pe_path:
                            nc.scalar.activation(
                                out=pts[i][:, 2 * mp:2 * mp + 2, :].rearrange(
                                    "p a b -> p (a b)"),
                                in_=sps[i], func=AF.Exp, scale=0.125)
                        else:
                            et = pet.tile([128, 1024], BF, tag="et",
                                          name=f"et_{hp}_{mp}_{i}")
                            nc.scalar.activation(out=et, in_=sps[i],
                                                 func=AF.Exp, scale=0.125)
                            nc.vector.tensor_mul(
                                out=pts[i][:, 2 * mp:2 * mp + 2, :].rearrange(
                                    "p a b -> p (a b)"),
                                in0=et,
                                in1=expb_s[:, 2 * mp:2 * mp + 2, :].rearrange(
                                    "p a b -> p (a b)"))
                for i in range(2):
                    hh = 2 * hp + i
                    cp = pdps2.tile([DH + 1, SQ], F32, tag="cx",
                                    name=f"cp_{hh}")
                    for mc in range(MC):
                        nc.tensor.matmul(cp, vaug_s[:, mc, hh, :],
                                         pts[i][:, mc, :],
                                         start=(mc == 0), stop=(mc == MC - 1))
                    cs = pd.tile([DH + 1, SQ], BF, tag="cs", name=f"cs_{hh}")
                    nc.vector.tensor_copy(out=cs, in_=cp)
                    for lc in range(LC):
                        tp = pdpst.tile([128, DH + 1], BF, tag="ct",
                                        name=f"ct_{hh}_{lc}")
                        nc.tensor.transpose(tp, cs[:, lc * 128:(lc + 1) * 128],
                                            ident_b[:DH + 1, :DH + 1])
                        rc = pd.tile([128, 1], F32, tag="rc",
                                     name=f"rc_{hh}_{lc}")
                        nc.vector.reciprocal(out=rc, in_=tp[:, DH:DH + 1])
                        nc.vector.tensor_scalar_mul(
                            out=ctxn_s[:, lc, hh * DH:(hh + 1) * DH],
                            in0=tp[:, 0:DH], scalar1=rc)

        es_attn.close()  # free expb/QT/KT/V

        # ================= Phase E: residual + LN =================
        es_x = ExitStack()        # phases E..G
        p_x = es_x.enter_context(tc.tile_pool(name="p_x", bufs=1))
        xh_s = p_x.tile([128, LC, H], F32)             # standardized x
        x2_s = p_x.tile([128, LC, H], F32)             # xh*g + (ln_b + bf2)
        xT_s = p_x.tile([128, KO, SQ], BF)             # xh^T
        gT_s = p_x.tile([128, IC, SQ], BF)             # gelu(ffn1)^T
        with tc.tile_pool(name="ph_e", bufs=2) as pe, \
             tc.tile_pool(name="ph_e_ps", bufs=4, space="PSUM") as peps:
            for lc in range(LC):
                xs = xh_s[:, lc, :]
                nc.vector.tensor_add(out=xs, in0=hres_s[:, lc, :],
                                     in1=ctxn_s[:, lc, :])
                st = pe.tile([128, 5, 6], F32, tag="st")
                xg = xs.rearrange("p (g d) -> p g d", d=256)
                for sg in range(5):
                    nc.vector.bn_stats(out=st[:, sg, :], in_=xg[:, sg, :])
                mv = pe.tile([128, 2], F32, tag="mv")
                nc.vector.bn_aggr(out=mv, in_=st)
                sd = pe.tile([128, 1], F32, tag="sd")
                nc.scalar.activation(out=sd, in_=mv[:, 1:2], func=AF.Sqrt,
                                     bias=eps_s)
                rs = pe.tile([128, 1], F32, tag="rs")
                nc.vector.reciprocal(out=rs, in_=sd)
                nc.vector.tensor_scalar(out=xs, in0=xs, scalar1=mv[:, 0:1],
                                        scalar2=rs, op0=ALU.subtract, op1=ALU.mult)
                # xg = xh*g on gpsimd (idle here); lnbf added in phase G
                nc.gpsimd.tensor_mul(out=x2_s[:, lc, :], in0=xs, in1=lng_b)
                for ko in range(KO):
                    tpx = peps.tile([128, 128], BF, tag="xt")
                    xsb = pe.tile([128, 128], BF, tag="xsb")
                    nc.scalar.copy(out=xsb, in_=xs[:, ko * 128:(ko + 1) * 128])
                    nc.tensor.transpose(tpx, xsb, ident_b)
                    nc.vector.tensor_copy(
                        out=xT_s[:, ko, lc * 128:(lc + 1) * 128], in_=tpx)
        es_ctx.close()  # free ctxn

        # ================= Phase F: FFN1 (gelu) =================
        with tc.tile_pool(name="ph_f_w", bufs=8) as pfw, \
             tc.tile_pool(name="ph_f_ps", bufs=4, space="PSUM") as pfps:
            for ic2 in range(IC // 2):
                wt = pfw.tile([128, KO, 256], BF, tag="w1")
                eng = (nc.sync, nc.gpsimd, nc.scalar)[ic2 % 3]
                eng.dma_start(out=wt, in_=wf1T_d.ap()[ic2])
                for i_in in range(2):
                    ic = ic2 * 2 + i_in
                    yp = pfps.tile([128, SQ], F32, tag="y")
                    for ko in range(KO):
                        nc.tensor.matmul(yp,
                                         wt[:, ko, i_in * 128:(i_in + 1) * 128],
                                         xT_s[:, ko, :],
                                         start=(ko == 0), stop=(ko == KO - 1))
                    nc.scalar.activation(out=gT_s[:, ic, :], in_=yp, func=AF.Gelu,
                                         bias=bf1_s[:, ic:ic + 1])

        # ================= Phase G: FFN2 + residual + store =================
        with tc.tile_pool(name="ph_g_w", bufs=8) as pgw, \
             tc.tile_pool(name="ph_g_o", bufs=3) as pgo, \
             tc.tile_pool(name="ph_g_ps", bufs=2, space="PSUM") as pgps:
            for j0, jn in ((0, 512), (512, 512), (1024, 256)):
                zps = [pgps.tile([128, jn], F32, tag=f"z{lc}", name=f"zp_{j0}_{lc}")
                       for lc in range(LC)]
                for ic in range(IC):
                    w2 = pgw.tile([128, 512], BF, tag="w2")
                    eng = (nc.sync, nc.gpsimd, nc.scalar)[ic % 3]
                    eng.dma_start(out=w2[:, :jn],
                                  in_=wf2T_d.ap()[ic, :, j0:j0 + jn])
                    for lc in range(LC):
                        nc.tensor.matmul(zps[lc],
                                         gT_s[:, ic, lc * 128:(lc + 1) * 128],
                                         w2[:, :jn],
                                         start=(ic == 0), stop=(ic == IC - 1))
                for lc in range(LC):
                    ot = pgo.tile([128, 512], F32, tag="ot")
                    nc.vector.tensor_add(out=ot[:, :jn], in0=zps[lc],
                                         in1=x2_s[:, lc, j0:j0 + jn])
                    nc.vector.tensor_add(out=ot[:, :jn], in0=ot[:, :jn],
                                         in1=lnbf_b[:, j0:j0 + jn])
                    nc.sync.dma_start(
                        out=out_d.ap()[lc * 128:(lc + 1) * 128, j0:j0 + jn],
                        in_=ot[:, :jn])
        es_x.close()

    nc.compile()
    return nc


_NC_CACHE = {}


def _get_nc(use_mask: bool, bias_scale: float):
    key = (use_mask, round(bias_scale, 9))
    if key not in _NC_CACHE:
        _NC_CACHE[key] = build_nc(use_mask, bias_scale)
    return _NC_CACHE[key]


def _prep_inputs(inputs):
    f32 = lambda x: np.ascontiguousarray(np.asarray(x, np.float32))
    hs = f32(inputs["hidden_states"])
    mask = f32(inputs["attention_mask"])
    M, W1, b1, W2, b2 = (f32(inputs["M"]), f32(inputs["W_ct1"]),
                         f32(inputs["b_ct1"]), f32(inputs["W_ct2"]),
                         f32(inputs["b_ct2"]))
    R = ((M.T @ W1.T + b1).T @ (M @ W2.T + b2)).astype(np.float32)
    bias_scale = float(np.asarray(inputs["bias_scale"]).reshape(-1)[0])
    use_mask = not bool(np.all(mask == 1.0))

    def pack_fp8_pairs(wT, m_block):
        # (H, J) -> [128, J/m_block, KO2, 2, m_block] fp8, scaled by WSCALE
        Hh, J = wT.shape
        w = (wT * WSCALE).reshape(KO2, 2, 128, J // m_block, m_block)
        return np.ascontiguousarray(w.transpose(2, 3, 0, 1, 4)).astype(FP8)

    wqT, wkT, wvT = (f32(inputs["Wq"]).T, f32(inputs["Wk"]).T,
                     f32(inputs["Wv"]).T)
    wq8 = pack_fp8_pairs(wqT, 128)                # (128, 10, 5, 2, 128)
    wk8 = pack_fp8_pairs(wkT, 128)
    # V moving: [p, j, i, vh]
    wv8 = np.ascontiguousarray(
        (wvT * WSCALE).reshape(KO2, 2, 128, H).transpose(2, 0, 1, 3)).astype(FP8)
    wptmT = np.zeros((H, 128), np.float32)
    wptmT[:, :P] = f32(inputs["W_ptm"]).T
    wptm8 = pack_fp8_pairs(wptmT, 128)[:, 0]      # (128, 5, 2, 128), zero-padded

    lng = f32(inputs["ln_g"])
    lnb = f32(inputs["ln_b"])
    Wf1 = f32(inputs["Wf1"])
    wf1T = np.ascontiguousarray((Wf1 * lng[None, :]).T).astype(BF16)  # (H, I)
    wf1p = np.ascontiguousarray(
        wf1T.reshape(KO, 128, IC // 2, 256).transpose(2, 0, 1, 3)
            .transpose(0, 2, 1, 3))               # (IC//2, 128, KO, 256)
    wf2T = np.ascontiguousarray(f32(inputs["Wf2"]).T).astype(BF16)    # (I, H)
    wf2p = np.ascontiguousarray(wf2T.reshape(IC, 128, H))
    shared = {
        "wq8": wq8, "wk8": wk8, "wv8": wv8, "wptm8": wptm8,
        "rmat": np.ascontiguousarray(R).astype(BF16),
        "wf1T": wf1p, "wf2T": wf2p,
        "bq": np.ascontiguousarray(f32(inputs["bq"]).reshape(KO, 128).T),
        "bk": np.ascontiguousarray(f32(inputs["bk"]).reshape(KO, 128).T),
        "bptm": f32(inputs["b_ptm"]).reshape(P, 1),
        "bf1": np.ascontiguousarray(
            (f32(inputs["bf1"]) + Wf1 @ lnb).reshape(IC, 128).T),
        "lng": np.ascontiguousarray(np.broadcast_to(lng, (128, H))),
        "lnbf": np.ascontiguousarray(
            np.broadcast_to(lnb + f32(inputs["bf2"]), (128, H))),
    }
    bv = f32(inputs["bv"])
    in_maps = []
    for c in range(8):
        b, half = c // 2, c % 2
        r0 = half * SQ
        mb = np.roll((1.0 - mask[b]) * np.float32(-1e30), -r0)
        m = dict(shared)
        m["h"] = np.ascontiguousarray(np.roll(hs[b], -r0, axis=0)).astype(FP8)
        m["hres"] = np.ascontiguousarray(hs[b, r0:r0 + SQ] + bv[None, :])
        m["mb"] = np.ascontiguousarray(mb.reshape(MC, 128).T)
        in_maps.append(m)
    return in_maps, use_mask, bias_scale


def kernel(**inputs) -> np.ndarray:
    in_maps, use_mask, bias_scale = _prep_inputs(inputs)
    nc = _get_nc(use_mask, bias_scale)
    res = run_bass_kernel_spmd(nc, in_maps, list(range(8)))
    out = np.zeros((B, S, H), np.float32)
    for c in range(8):
        b, half = c // 2, c % 2
        r0 = half * SQ
        out[b, r0:r0 + SQ] = res.results[c]["out"]
    return out
